# revision 62
# baseline (speedup 1.0000x reference)
"""Trainium2 Bass kernel for nn_MiddleLayerEncoder (gnn_message_passing).

Strategy: shard by CLUSTER across the 8 cores (each core owns 512 whole
clusters and all their points), so both segment_max reductions are
core-local and no collectives are needed.

v3 design: points sorted by cluster, padded per-rank to canonical size
L (multiple of 4, min 36, identical across cores -> one SPMD program),
grouped into chunks of <= 61 ranks / <= 4096 columns.  Both segment_max
stages are fused into PSUM evacuation: vector reduce_max with a 3D
access pattern reads matmul outputs straight out of PSUM per cluster
run -- no copies, no tree reductions, no intermediate buffers.  PSUM
windows are aligned to rank boundaries so each reduce covers whole
clusters.  The neigh_enc[cluster] gather is fused into the W1 matmul
via per-chunk one-hot rows (rank-in-chunk at partitions 64:125), so a
single per-chunk lhsT = [W1A | M | W1B] serves every l1 matmul of the
chunk; M = neighT^T @ W1c is placed at PSUM partitions 64:125 by the
matmul itself and evacuated in place.
"""

import numpy as np
import ml_dtypes
from contextlib import ExitStack

import concourse.bass as bass
import concourse.bacc as bacc
import concourse.tile as tile
from concourse import mybir
from concourse.bass_utils import run_bass_kernel_spmd

BF16 = mybir.dt.bfloat16
F32 = mybir.dt.float32
NPBF16 = ml_dtypes.bfloat16

N_CORES = 8
N_PTS = 262144
N_CLUSTERS = 4096
MINL = 36          # minimum padded points per cluster (multiple of 4)
MAX_RANKS = 61     # chunk rank cap: 64 + 61 + 3 = 128 lhsT partitions
MAX_COLS = 4096    # chunk column cap (SBUF tiling)
WIN = 1024         # PSUM window (2 banks)

# l1 evacuation engine per window, round-robin (S=scalar, V=vector)
L1_EVAC = "S"

# packed bf16 weight slots: name -> (col0, rows)
WSLOT = {
    "enc1_lhsT": (0, (0, 12)),
    "enc2_lhsT": (128, (0, 64)),
    "enc2_hi": (256, (64, 128)),
    "W1A": (384, (0, 64)),
    "W1B": (512, (125, 128)),
    "W1c": (640, (0, 64)),
    "fcW2": (768, (0, 128)),
    "G1": (896, (0, 128)),
    "G2a": (1024, (0, 128)),
    "G2b": (1152, (0, 128)),
}
WCOLS = 1280
# packed f32 bias slots: name -> col
BSLOT = {"b_enc1_4": 0, "b_enc2": 1, "b1": 2, "b2": 3,
         "gb1": 4, "gb2a": 5, "gb2b": 6}
BCOLS = 7


# ---------------------------------------------------------------- planning

def _plan(cluster):
    """Canonical SPMD layout shared by all cores."""
    counts = np.bincount(cluster, minlength=N_CLUSTERS)
    assert counts.min() >= 1, "empty cluster unsupported"
    order = np.argsort(-counts, kind="stable")  # cluster ids, size desc

    # snake-deal into N_CORES bins -> per-core 512 clusters, balanced sizes
    n_ranks = N_CLUSTERS // N_CORES
    cids = np.empty((N_CORES, n_ranks), dtype=np.int64)
    for i, cid in enumerate(order):
        rnd, pos = divmod(i, N_CORES)
        core = pos if rnd % 2 == 0 else N_CORES - 1 - pos
        cids[core, rnd] = cid

    sizes = counts[cids]                      # [cores, ranks]
    L = sizes.max(axis=0)                     # canonical per-rank size
    L = np.maximum((L + 3) // 4 * 4, MINL).astype(np.int64)

    col0 = np.concatenate([[0], np.cumsum(L)])  # rank -> start col
    S = int(col0[-1])

    def runs(r0, r1):
        """class runs [(i, j)] of equal L within ranks [r0, r1)."""
        out = []
        i = r0
        while i < r1:
            j = i
            while j < r1 and L[j] == L[i]:
                j += 1
            out.append((i, j))
            i = j
        return out

    def mk_chunk(r0, r1):
        c0, cc = int(col0[r0]), int(col0[r1] - col0[r0])
        # rank-aligned PSUM windows of <= WIN cols
        wins = []
        i = r0
        while i < r1:
            j = i
            while j < r1 and col0[j + 1] - col0[i] <= WIN:
                j += 1
            wins.append(dict(
                i=i, j=j,
                off=int(col0[i] - col0[r0]),          # chunk-local col
                wcc=int(col0[j] - col0[i]),
                classes=[(a, b, int(col0[a] - col0[i])) for a, b in runs(i, j)],
            ))
            i = j
        return dict(
            r0=r0, r1=r1, c0=c0, cc=cc, wins=wins,
            classes=[(a, b, int(col0[a] - col0[r0])) for a, b in runs(r0, r1)],
        )

    # chunks: whole clusters, <= MAX_COLS cols, <= MAX_RANKS ranks
    chunks = []
    r0 = 0
    while r0 < n_ranks:
        r1 = r0
        while (
            r1 < n_ranks
            and (col0[r1 + 1] - col0[r0]) <= MAX_COLS
            and (r1 - r0) < MAX_RANKS
        ):
            r1 += 1
        chunks.append(mk_chunk(r0, r1))
        r0 = r1




    # processing order: smallest chunk first (fast pipeline fill) and
    # second-smallest last (fast drain); big chunks in the middle
    idx = sorted(range(len(chunks)), key=lambda i: chunks[i]["cc"])
    if len(chunks) > 2:
        mids = sorted(set(range(len(chunks))) - {idx[0], idx[1]})
        corder = [idx[0]] + mids + [idx[1]]
    else:
        corder = list(range(len(chunks)))

    return dict(cids=cids, sizes=sizes, L=L, col0=col0, S=S, chunks=chunks,
                corder=corder, n_ranks=n_ranks)


def _prep_core(k, plan, rel_points, features, cluster, sort_idx, bucket0):
    """Per-core input arrays (canonical layout, core-specific data)."""
    L, col0, S = plan["L"], plan["col0"], plan["S"]
    cids = plan["cids"][k]
    n_ranks = plan["n_ranks"]

    slot = np.empty(S, dtype=np.int64)
    oh_row = np.empty(S, dtype=np.int64)   # one-hot row: 64 + rank-in-chunk
    for ck in plan["chunks"]:
        for r in range(ck["r0"], ck["r1"]):
            cid = cids[r]
            idx = sort_idx[bucket0[cid]: bucket0[cid + 1]]
            n = idx.shape[0]
            c0, c1 = int(col0[r]), int(col0[r + 1])
            slot[c0: c0 + n] = idx
            if c1 - c0 > n:
                slot[c0 + n: c1] = idx[0]
            oh_row[c0:c1] = 64 + (r - ck["r0"])

    pts = rel_points[slot]          # [S, 3] f32
    feat = features[slot]           # [S, 64] f32

    # rows 0:64 = pts + feat[0:61] (= W1[0:64] order)
    # rows 64:125 = one-hot rank-in-chunk; rows 125:128 = feat[61:64]
    encT = np.zeros((128, S), dtype=NPBF16)
    encT[0:3] = pts.T.astype(NPBF16)
    encT[3:64] = feat.T[0:61].astype(NPBF16)
    encT[125:128] = feat.T[61:64].astype(NPBF16)
    encT[oh_row, np.arange(S)] = NPBF16(1.0)

    # pts4: [12, S/4] quad-packed points
    pts4 = (
        pts.astype(NPBF16)
        .reshape(S // 4, 4, 3)
        .transpose(1, 2, 0)
        .reshape(12, S // 4)
    )
    pts4 = np.ascontiguousarray(pts4)
    return {"encT": encT, "pts4": pts4}


def _blockdiag(w, times):
    fi, fo = w.shape
    out = np.zeros((fi * times, fo * times), dtype=w.dtype)
    for i in range(times):
        out[i * fi:(i + 1) * fi, i * fo:(i + 1) * fo] = w
    return out


def _prep_weights(inp):
    W1 = inp["W1"]
    mats = {
        "enc1_lhsT": _blockdiag(inp["enc_W1"], 4),
        "enc2_lhsT": _blockdiag(inp["enc_W2"], 2),
        "enc2_hi": _blockdiag(inp["enc_W2"], 2),
        "W1A": W1[0:64],
        "W1B": W1[64:67],
        "W1c": W1[67:131],
        "fcW2": inp["W2"],
        "G1": inp["G1"],
        "G2a": inp["G2"][:, 0:128],
        "G2b": inp["G2"][:, 128:256],
    }
    wpack = np.zeros((128, WCOLS), dtype=NPBF16)
    for name, (c, (p0, p1)) in WSLOT.items():
        m = mats[name]
        assert m.shape[0] == p1 - p0, name
        wpack[p0:p1, c:c + m.shape[1]] = m.astype(NPBF16)

    bias = {
        "b_enc1_4": np.tile(inp["enc_b1"], 4),
        "b_enc2": np.pad(inp["enc_b2"], (0, 64)),
        "b1": inp["b1"],
        "b2": inp["b2"],
        "gb1": inp["gb1"],
        "gb2a": inp["gb2"][0:128],
        "gb2b": inp["gb2"][128:256],
    }
    bpack = np.zeros((128, BCOLS), dtype=np.float32)
    for name, c in BSLOT.items():
        bpack[:, c] = bias[name]
    return {"wpack": wpack, "bpack": bpack}


# ---------------------------------------------------------------- program

def _build(plan):
    S = plan["S"]
    L = plan["L"]
    n_ranks = plan["n_ranks"]
    chunks = [plan["chunks"][i] for i in plan["corder"]]
    n_chunks = len(chunks)
    nc = bacc.Bacc(None, target_bir_lowering=False, debug=True)

    encT_d = nc.dram_tensor("encT", [128, S], BF16, kind="ExternalInput")
    pts4_d = nc.dram_tensor("pts4", [12, S // 4], BF16, kind="ExternalInput")
    wpack_d = nc.dram_tensor("wpack", [128, WCOLS], BF16, kind="ExternalInput")
    bpack_d = nc.dram_tensor("bpack", [128, BCOLS], F32, kind="ExternalInput")
    out_d = nc.dram_tensor("out", [256, 512], F32, kind="ExternalOutput")

    RELU = mybir.ActivationFunctionType.Relu
    COPY = mybir.ActivationFunctionType.Copy
    MAX = mybir.AluOpType.max
    ADD = mybir.AluOpType.add
    AXX = mybir.AxisListType.X

    with tile.TileContext(nc) as tc, ExitStack() as ctx:
        consts = ctx.enter_context(tc.tile_pool(name="consts", bufs=1))
        glob = ctx.enter_context(tc.tile_pool(name="glob", bufs=1))
        stream = ctx.enter_context(tc.tile_pool(name="stream", bufs=3))
        mid = ctx.enter_context(tc.tile_pool(name="mid", bufs=2))
        small = ctx.enter_context(tc.tile_pool(name="small", bufs=3))
        ps = ctx.enter_context(tc.tile_pool(name="ps", bufs=4, space="PSUM"))

        wp = consts.tile([128, WCOLS], BF16, tag="wpack")
        bp = consts.tile([128, BCOLS], F32, tag="bpack")

        def W(name):
            c, (p0, p1) = WSLOT[name]
            return wp[p0:p1, c:c + 128]

        def B(name):
            return bp[:, BSLOT[name]:BSLOT[name] + 1]

        def B64(name):
            return bp[0:64, BSLOT[name]:BSLOT[name] + 1]

        pre_neigh = glob.tile([128, n_ranks], BF16, tag="pre_neigh")
        T2buf = glob.tile([128, n_ranks], BF16, tag="T2buf")
        nT = glob.tile([64, 128], BF16, tag="nT")
        lhsT_bufs = []
        for i in range(3):
            lb = glob.tile([128, 128], BF16, tag=f"lhsT{i}", name=f"lhsT{i}")
            lhsT_bufs.append(lb)

        enc_tiles = {}   # chunk -> encT_t
        lhsT_of = {}     # chunk -> lhsT buffer
        l1_widx = [0]

        gT = glob.tile([128, n_ranks], BF16, tag="gT")
        g1T = glob.tile([128, n_ranks], BF16, tag="g1T")
        osbA = glob.tile([128, n_ranks], F32, tag="osbA")
        osbB = glob.tile([128, n_ranks], F32, tag="osbB")

        def stage_G(k):
            """Global MLP on this chunk's T2 columns (column-parallel)."""
            ck = chunks[k]
            r0, r1 = ck["r0"], ck["r1"]
            nk = r1 - r0
            nc.scalar.activation(gT[:, r0:r1], T2buf[:, r0:r1], RELU,
                                 bias=B("b2"), scale=1.0)
            pg = ps.tile([128, WIN], F32, tag="ps")
            nc.tensor.matmul(pg[:, 0:nk], W("G1"), gT[:, r0:r1],
                             start=True, stop=True)
            nc.scalar.activation(g1T[:, r0:r1], pg[:, 0:nk], RELU,
                                 bias=B("gb1"), scale=1.0)
            po = ps.tile([128, WIN], F32, tag="ps")
            nc.tensor.matmul(po[:, 0:nk], W("G2a"), g1T[:, r0:r1],
                             start=True, stop=True)
            nc.tensor.matmul(po[:, 512:512 + nk], W("G2b"), g1T[:, r0:r1],
                             start=True, stop=True)
            nc.scalar.activation(osbA[:, r0:r1], po[:, 0:nk], RELU,
                                 bias=B("gb2a"), scale=1.0)
            nc.scalar.activation(osbB[:, r0:r1], po[:, 512:512 + nk], RELU,
                                 bias=B("gb2b"), scale=1.0)

        def stage_A_dma(k):
            ck = chunks[k]
            c0, cc = ck["c0"], ck["cc"]
            pts4_t = mid.tile([12, MAX_COLS // 4], BF16, tag="pts4_t", bufs=3)
            nc.gpsimd.dma_start(out=pts4_t[:, :cc // 4],
                                in_=pts4_d[:, c0 // 4: c0 // 4 + cc // 4])
            encT_t = stream.tile([128, MAX_COLS], BF16, tag="encT_t", bufs=5)
            h = cc // 2
            nc.sync.dma_start(out=encT_t[:, :h], in_=encT_d[:, c0:c0 + h])
            nc.sync.dma_start(out=encT_t[:, h:cc],
                              in_=encT_d[:, c0 + h:c0 + cc])
            enc_tiles[k] = (encT_t, pts4_t)

        def stage_A_enc1(k):
            """enc1: [12 -> 128] on quad-packed points.

            h1 is evacuated in 512-col pieces so the first enc2 matmul can
            start as soon as the first piece lands (PE stall shrinks).
            """
            ck = chunks[k]
            cq = ck["cc"] // 4
            (_, pts4_t) = enc_tiles[k]
            pe1 = ps.tile([128, WIN], F32, tag="ps")
            for s in range(0, cq, 512):
                w = min(512, cq - s)
                nc.tensor.matmul(pe1[:, s:s + w], W("enc1_lhsT"),
                                 pts4_t[:, s:s + w], start=True, stop=True)
            h1 = mid.tile([128, WIN], BF16, tag="h1")
            nc.scalar.activation(h1[:, :cq], pe1[:, :cq], RELU,
                                 bias=B("b_enc1_4"), scale=1.0)
            return h1

        def stage_A_enc2(k, h1):
            """enc2: pA (pts 0,1), pB (pts 2,3); seg1 via direct PSUM reduce."""
            ck = chunks[k]
            r0, r1, cc = ck["r0"], ck["r1"], ck["cc"]
            cq = cc // 4
            nk = r1 - r0
            pe2a = ps.tile([128, WIN], F32, tag="ps")
            pe2b = ps.tile([128, WIN], F32, tag="ps")
            for s in range(0, cq, 512):
                w = min(512, cq - s)
                nc.tensor.matmul(pe2a[:, s:s + w], W("enc2_lhsT"),
                                 h1[0:64, s:s + w], start=True, stop=True)
                nc.tensor.matmul(pe2b[:, s:s + w], W("enc2_hi"),
                                 h1[64:128, s:s + w], start=True, stop=True)
            TA = small.tile([128, 64], BF16, tag="TA")
            TB = small.tile([128, 64], BF16, tag="TB")
            for (a, b, off) in ck["classes"]:
                n = b - a
                w4 = int(L[a]) // 4
                q0 = off // 4
                va = pe2a[:, q0: q0 + n * w4].rearrange("p (n w) -> p n w", w=w4)
                vb = pe2b[:, q0: q0 + n * w4].rearrange("p (n w) -> p n w", w=w4)
                nc.vector.reduce_max(TA[:, a - r0: b - r0], va, axis=AXX)
                nc.vector.reduce_max(TB[:, a - r0: b - r0], vb, axis=AXX)
            nc.vector.tensor_max(pre_neigh[:, r0:r1], TA[:, :nk], TB[:, :nk])
            # fold 128 -> 64 partitions (issued here; consumed in stage_B)
            fold = small.tile([64, 64], BF16, tag="fold")
            nc.sync.dma_start(out=fold[:, :nk], in_=pre_neigh[64:128, r0:r1])
            return fold

        def stage_A(k):
            return stage_A_enc2(k, stage_A_enc1(k))

        def stage_B_mx(k, fold):
            """fold-max on vector (early: fold DMA is one iteration old)."""
            ck = chunks[k]
            r0, r1 = ck["r0"], ck["r1"]
            nk = r1 - r0
            mx = small.tile([64, 64], BF16, tag="mx")
            nc.vector.tensor_max(mx[:, :nk], pre_neigh[0:64, r0:r1],
                                 fold[:, :nk])
            return mx

        def stage_B_nT(k, mx):
            """neighT relu+bias (scalar, after l1 evacuations)."""
            ck = chunks[k]
            nk = ck["r1"] - ck["r0"]
            nc.scalar.activation(nT[:, 64:64 + nk], mx[:, :nk], RELU,
                                 bias=B64("b_enc2"), scale=1.0)

        def stage_B_pm(k):
            """M = nT^T @ W1c -> per-chunk l1 lhsT (PE tail of iteration)."""
            pm = ps.tile([128, WIN], F32, tag="ps")
            nc.tensor.matmul(pm[:, 0:128], nT[:], W("W1c"),
                             start=True, stop=True)
            lhsT_k = lhsT_bufs[k % 3]
            nc.scalar.activation(lhsT_k[64:125, :], pm[64:125, 0:128], COPY)
            lhsT_of[k] = lhsT_k

        def stage_C_l1(k):
            """l1 matmuls + relu evacuation -> e1."""
            ck = chunks[k]
            (encT_t, _) = enc_tiles.pop(k)
            lhsT_k = lhsT_of.pop(k)

            e1 = stream.tile([128, MAX_COLS], BF16, tag="e1", bufs=2)
            for win in ck["wins"]:
                off, wcc = win["off"], win["wcc"]
                p = ps.tile([128, WIN], F32, tag="ps")
                for s in range(0, wcc, 512):
                    w = min(512, wcc - s)
                    nc.tensor.matmul(p[:, s:s + w], lhsT_k[:],
                                     encT_t[:, off + s:off + s + w],
                                     start=True, stop=True)
                eng = L1_EVAC[l1_widx[0] % len(L1_EVAC)]
                l1_widx[0] += 1
                if eng == "S":
                    nc.scalar.activation(e1[:, off:off + wcc], p[:, :wcc],
                                         RELU, bias=B("b1"), scale=1.0)
                else:
                    nc.vector.tensor_scalar(
                        e1[:, off:off + wcc], p[:, :wcc], B("b1"), 0.0,
                        op0=ADD, op1=MAX,
                    )
            return e1

        def stage_C_l2(k, e1):
            """l2 matmuls; seg2 fused into PSUM reduce -> T2."""
            ck = chunks[k]
            for win in ck["wins"]:
                off, wcc = win["off"], win["wcc"]
                p = ps.tile([128, WIN], F32, tag="ps")
                for s in range(0, wcc, 512):
                    w = min(512, wcc - s)
                    nc.tensor.matmul(p[:, s:s + w], W("fcW2"),
                                     e1[:, off + s:off + s + w],
                                     start=True, stop=True)
                for (a, b, woff) in win["classes"]:
                    n = b - a
                    wL = int(L[a])
                    v = p[:, woff: woff + n * wL].rearrange(
                        "p (n w) -> p n w", w=wL)
                    nc.vector.reduce_max(T2buf[:, a:b], v, axis=AXX)

        # ---- pipeline: C_l1(k) | A(k+2) | B(k+1) | C_l2(k)
        nc.sync.dma_start(out=wp[:], in_=wpack_d[:])
        nc.sync.dma_start(out=bp[:], in_=bpack_d[:])
        for j in range(min(4, n_chunks)):
            stage_A_dma(j)
        nc.vector.memset(nT[:], 0.0)
        # dummy relu: loads the scalar ACT table during the DMA phase
        warm = small.tile([64, 1], F32, tag="warm")
        nc.scalar.activation(warm[:], nT[:, 0:1], RELU, scale=1.0)
        for i, lb in enumerate(lhsT_bufs):
            nc.scalar.dma_start(out=lb[0:64, :],
                                in_=wpack_d[0:64, WSLOT["W1A"][0]:
                                            WSLOT["W1A"][0] + 128])
            nc.scalar.dma_start(out=lb[125:128, :],
                                in_=wpack_d[125:128, WSLOT["W1B"][0]:
                                            WSLOT["W1B"][0] + 128])
        folds = {}
        h1s = {}
        fold0 = stage_A(0)
        if n_chunks > 1:
            folds[1] = stage_A(1)
        if n_chunks > 2:
            h1s[2] = stage_A_enc1(2)
        mx0 = stage_B_mx(0, fold0)
        stage_B_nT(0, mx0)
        stage_B_pm(0)
        e1s = {}
        for k in range(n_chunks):
            if k + 4 < n_chunks:
                stage_A_dma(k + 4)
            if k + 3 < n_chunks:
                h1s[k + 3] = stage_A_enc1(k + 3)
            if k + 2 < n_chunks:
                folds[k + 2] = stage_A_enc2(k + 2, h1s.pop(k + 2))
            if k + 1 < n_chunks:
                mxk = stage_B_mx(k + 1, folds.pop(k + 1))
            e1s[k] = stage_C_l1(k)
            if k + 1 < n_chunks:
                stage_B_nT(k + 1, mxk)
            if k > 0:
                stage_C_l2(k - 1, e1s.pop(k - 1))
            if k + 1 < n_chunks:
                stage_B_pm(k + 1)
            if k > 1:
                stage_G(k - 2)
        if n_chunks > 1:
            stage_G(n_chunks - 2)
        stage_C_l2(n_chunks - 1, e1s.pop(n_chunks - 1))
        stage_G(n_chunks - 1)

        nc.sync.dma_start(out=out_d[0:128, :], in_=osbA[:])
        nc.sync.dma_start(out=out_d[128:256, :], in_=osbB[:])

    nc.finalize()
    return nc


# ---------------------------------------------------------------- entry

_CACHE = {}


def _run(inputs, trace=False, **spmd_kwargs):
    cluster = np.asarray(inputs["cluster"])
    key = hash(cluster.tobytes())
    if key not in _CACHE:
        plan = _plan(cluster)
        nc = _build(plan)
        _CACHE[key] = (plan, nc)
    plan, nc = _CACHE[key]

    rel_points = np.asarray(inputs["relative_points"], dtype=np.float32)
    features = np.asarray(inputs["features"], dtype=np.float32)
    sort_idx = np.argsort(cluster, kind="stable")
    bucket0 = np.concatenate(
        [[0], np.cumsum(np.bincount(cluster, minlength=N_CLUSTERS))]
    )
    wmap = _prep_weights({k: np.asarray(v, dtype=np.float32)
                          for k, v in inputs.items()
                          if k not in ("relative_points", "features", "cluster")})

    in_maps = []
    for k in range(N_CORES):
        m = _prep_core(k, plan, rel_points, features, cluster, sort_idx, bucket0)
        m.update(wmap)
        in_maps.append(m)

    res = run_bass_kernel_spmd(nc, in_maps, list(range(N_CORES)),
                               trace=trace, **spmd_kwargs)

    out = np.empty((N_CLUSTERS, 256), dtype=np.float32)
    for k in range(N_CORES):
        out[plan["cids"][k]] = res.results[k]["out"].T
    return out, res


def kernel(**inputs):
    return _run(inputs)[0]
